# revision 1
# baseline (speedup 1.0000x reference)
"""Trainium2 Bass kernel: attention with relative-position bias.

Reference computation (per sequence, B*T=16 sequences of L=512, D=1024):
    qkv = x @ w_qkv;  q,k,v split;  S = q k^T / sqrt(dh) + rel_bias
    P = softmax(S);   out = (P @ v) @ w_out + b_out

Sharding: data-parallel over the B*T axis — 2 sequences per NeuronCore,
weights replicated. No collectives.

Per-core kernel (all matmuls fp16, accumulation fp32 in PSUM):
  - host pre-transposes x -> xT and pre-casts everything to fp16; the q
    columns of w_qkv are pre-scaled by dh^-0.5.
  - qkT = w_qk^T @ xT   (16 chunk tiles of [128, 512]; chunks 0-7 = q^T
    head-pairs, 8-15 = k^T head-pairs)
  - v   = xT^T @ w_v    (natural layout, stored with a 1.0 column appended
    per head: [128, 16*65] so the PV matmul also produces softmax sums)
  - S^T head-pair-packed: two K=64 matmuls concurrent via tile_position
    row tiling, accumulating into separate PSUM banks
  - P = exp(S^T) * expb  where expb = exp(rel_bias^T) is a host-precomputed
    skewed tile per head ([128, 896]; the s-chunk r bias tile is the slice
    [:, 384-128r : 896-128r] — the bias matrix is Toeplitz)
  - O^T|sums = v_aug^T @ P^T per head (M=65), normalize O^T rows by the
    broadcast reciprocal of the sums row
  - y^T = w_out^T @ O^T + b_out; host transposes back.

The per-sequence phases are software-pipelined at the source level:
sequence s+1's projections (A/B) are interleaved into sequence s's
attention (C), and s's output projection (D) into s+1's attention, so the
TensorE instruction stream has dense work while ACT/DVE run the softmax.
"""

import os
import numpy as np
import ml_dtypes

import concourse.bass as bass
import concourse.mybir as mybir
import concourse.tile as tile
from concourse import bacc, bass_utils

HEADS = 16
MAX_REL = 128
B, T, L, D = 2, 8, 512, 1024
DH = D // HEADS          # 64
N_CORES = 8
SEQS = B * T             # 16
SPC = SEQS // N_CORES    # sequences per core = 2
KC = D // 128            # contraction chunks = 8
LC = L // 128            # sequence chunks = 4
HP = HEADS // 2          # head pairs = 8
EXPB_W = 896             # skewed bias tile width (512 + 3*128)

_F32 = mybir.dt.float32
_F16 = mybir.dt.float16

LAST_EXEC_TIME_NS = None


def _build_program():
    nc = bacc.Bacc("TRN2", debug=False)

    # Per-core DRAM I/O (bf16 unless noted).
    xT_d = nc.dram_tensor("xT", [SPC, 128, KC, L], _F16, kind="ExternalInput")
    wqk_d = nc.dram_tensor("wqk", [16, 128, KC, 128], _F16, kind="ExternalInput")
    wv_d = nc.dram_tensor("wv", [2, 128, KC, 512], _F16, kind="ExternalInput")
    wo_d = nc.dram_tensor("wo", [KC, 128, 8, 128], _F16, kind="ExternalInput")
    expb_d = nc.dram_tensor("expb", [HEADS, 128, EXPB_W], _F16, kind="ExternalInput")
    bo_d = nc.dram_tensor("bo", [128, 8], _F32, kind="ExternalInput")
    yT_d = nc.dram_tensor("yT", [SPC, 128, 8, L], _F32, kind="ExternalOutput")

    with tile.TileContext(nc) as tc:
        with (
            tc.tile_pool(name="const", bufs=1) as const_pool,
            tc.tile_pool(name="wstream", bufs=4) as wstream,
            tc.tile_pool(name="xt", bufs=2) as xt_pool,
            tc.tile_pool(name="qkt", bufs=2) as qkt_pool,
            tc.tile_pool(name="vaug", bufs=2) as vaug_pool,
            tc.tile_pool(name="ptile", bufs=8) as p_pool,
            tc.tile_pool(name="ot", bufs=2) as ot_pool,
            tc.tile_pool(name="norm", bufs=3) as norm_pool,
            tc.tile_pool(name="ysb", bufs=3) as y_pool,
            tc.tile_pool(name="ps_mm", bufs=2, space="PSUM") as ps_mm,
            tc.tile_pool(name="ps_s", bufs=2, space="PSUM") as ps_s,
            tc.tile_pool(name="ps_o", bufs=1, space="PSUM") as ps_o,
        ):
            # ---- constants loaded once per core (SWDGE queue, off the
            # critical HWDGE path) ----
            expb_sb = const_pool.tile([128, HEADS, EXPB_W], _F16)
            nc.gpsimd.dma_start(
                out=expb_sb, in_=expb_d.ap().rearrange("h p u -> p h u")
            )
            wv_sb = const_pool.tile([128, 2, KC, 512], _F16)
            nc.gpsimd.dma_start(out=wv_sb, in_=wv_d.ap().rearrange("n p k c -> p n k c"))
            wo_sb = const_pool.tile([128, KC, 8, 128], _F16)
            nc.gpsimd.dma_start(out=wo_sb, in_=wo_d.ap().rearrange("i p m c -> p i m c"))
            bo_sb = const_pool.tile([128, 8], _F32)
            nc.gpsimd.dma_start(out=bo_sb, in_=bo_d.ap())

            # Per-sequence state (tiles), filled in by the phase generators.
            xt_sb = [None] * SPC
            qkt = [None] * SPC
            vaug = [None] * SPC
            ot = [None] * SPC

            def load_x(s):
                xt_sb[s] = xt_pool.tile([128, KC, L], _F16, name="xt", tag="xt")
                nc.sync.dma_start(out=xt_sb[s], in_=xT_d.ap()[s])

            def phase_a(s):
                """qk^T projection: 16 m-chunk steps."""
                qkt[s] = qkt_pool.tile([128, 16, L], _F16, name="qkt", tag="qkt")
                for m in range(16):
                    wqk_sb = wstream.tile([128, KC, 128], _F16, name="wqk", tag="wqk")
                    nc.sync.dma_start(out=wqk_sb, in_=wqk_d.ap()[m])
                    ps = ps_mm.tile([128, L], _F32, name="ps", tag="ps")
                    for k in range(KC):
                        nc.tensor.matmul(
                            ps,
                            wqk_sb[:, k, :],
                            xt_sb[s][:, k, :],
                            start=(k == 0),
                            stop=(k == KC - 1),
                        )
                    if m % 2 == 0:
                        nc.vector.tensor_copy(out=qkt[s][:, m, :], in_=ps)
                    else:
                        nc.scalar.activation(
                            out=qkt[s][:, m, :], in_=ps,
                            func=mybir.ActivationFunctionType.Copy,
                        )
                    yield

            def phase_b(s):
                """v projection: 8 (lc, nh) steps."""
                vaug[s] = vaug_pool.tile([128, LC, HEADS * 65], _F16, name="vaug", tag="vaug")
                va = vaug[s]
                for lc in range(LC):
                    ps0v = ps_mm.tile([128, 512], _F32, name="ps0v", tag="ps")
                    ps1v = ps_mm.tile([128, 512], _F32, name="ps1v", tag="ps")
                    for k in range(KC):
                        nc.tensor.matmul(
                            ps0v,
                            xt_sb[s][:, k, lc * 128:(lc + 1) * 128],
                            wv_sb[:, 0, k, :],
                            start=(k == 0),
                            stop=(k == KC - 1),
                        )
                        nc.tensor.matmul(
                            ps1v,
                            xt_sb[s][:, k, lc * 128:(lc + 1) * 128],
                            wv_sb[:, 1, k, :],
                            start=(k == 0),
                            stop=(k == KC - 1),
                        )
                    for nh, ps in ((0, ps0v), (1, ps1v)):
                        dst = bass.AP(
                            tensor=va.tensor,
                            offset=va.offset + lc * (HEADS * 65) + nh * 8 * 65,
                            ap=[va.ap[0], [65, 8], [1, 64]],
                        )
                        nc.vector.tensor_copy(
                            out=dst, in_=ps.rearrange("p (h c) -> p h c", h=8)
                        )
                    ones_dst = bass.AP(
                        tensor=va.tensor,
                        offset=va.offset + lc * (HEADS * 65) + 64,
                        ap=[va.ap[0], [65, HEADS], [1, 1]],
                    )
                    nc.vector.memset(ones_dst, 1.0)
                    yield
                    yield

            def phase_c(s):
                """attention: 8 head-pair steps."""
                ot[s] = ot_pool.tile([128, KC, L], _F16, name="ot", tag="ot")
                for hp in range(HP):
                    h0, h1 = 2 * hp, 2 * hp + 1
                    q_tile = qkt[s][:, hp, :]
                    k_tile = qkt[s][:, 8 + hp, :]
                    p_tiles = []
                    for r in range(LC):
                        ps0 = ps_s.tile([128, 512], _F32, name="s0", tag="s0")
                        ps1 = ps_s.tile([128, 512], _F32, name="s1", tag="s1")
                        nc.tensor.matmul(
                            ps0,
                            k_tile[0:64, r * 128:(r + 1) * 128],
                            q_tile[0:64, :],
                            start=True, stop=True,
                        )
                        nc.tensor.matmul(
                            ps1,
                            k_tile[64:128, r * 128:(r + 1) * 128],
                            q_tile[64:128, :],
                            start=True, stop=True,
                            tile_position=(64, 0),
                        )
                        p0 = p_pool.tile([128, 512], _F16, name="p0", tag="p0")
                        p1 = p_pool.tile([128, 512], _F16, name="p1", tag="p1")
                        off = 384 - 128 * r
                        nc.scalar.activation(
                            out=p0, in_=ps0, func=mybir.ActivationFunctionType.Exp
                        )
                        mul_eng = nc.gpsimd if r == 3 else nc.vector
                        mul_eng.tensor_mul(
                            out=p0, in0=p0, in1=expb_sb[:, h0, off:off + 512]
                        )
                        nc.scalar.activation(
                            out=p1, in_=ps1, func=mybir.ActivationFunctionType.Exp
                        )
                        mul_eng.tensor_mul(
                            out=p1, in0=p1, in1=expb_sb[:, h1, off:off + 512]
                        )
                        p_tiles.append((p0, p1))

                    po0 = ps_o.tile([65, 512], _F32, name="po0", tag="o0")
                    po1 = ps_o.tile([65, 512], _F32, name="po1", tag="o1")
                    for r in range(LC):
                        p0, p1 = p_tiles[r]
                        nc.tensor.matmul(
                            po0,
                            vaug[s][:, r, h0 * 65:h0 * 65 + 65],
                            p0,
                            start=(r == 0), stop=(r == LC - 1),
                        )
                        nc.tensor.matmul(
                            po1,
                            vaug[s][:, r, h1 * 65:h1 * 65 + 65],
                            p1,
                            start=(r == 0), stop=(r == LC - 1),
                        )
                    for idx, po in ((0, po0), (1, po1)):
                        rs_sb = norm_pool.tile([1, 512], _F32, name="rs_sb", tag="rs_sb")
                        nc.vector.tensor_copy(out=rs_sb, in_=po[64:65, :])
                        rsum = norm_pool.tile([1, 512], _F32, name="rsum", tag="rsum")
                        nc.vector.reciprocal_approx_fast(out=rsum, in_=rs_sb)
                        rb = norm_pool.tile([64, 512], _F32, name="rb", tag="rb")
                        nc.gpsimd.partition_broadcast(rb, rsum)
                        nc.vector.tensor_mul(
                            out=ot[s][idx * 64:(idx + 1) * 64, hp, :],
                            in0=po[0:64, :],
                            in1=rb,
                        )
                    yield

            def phase_d(s):
                """output projection: 8 m-chunk steps."""
                for m in range(8):
                    ps = ps_mm.tile([128, L], _F32, name="ps", tag="ps")
                    for i in range(KC):
                        nc.tensor.matmul(
                            ps,
                            wo_sb[:, i, m, :],
                            ot[s][:, i, :],
                            start=(i == 0),
                            stop=(i == KC - 1),
                        )
                    ysb = y_pool.tile([128, L], _F32, name="ysb", tag="ysb")
                    nc.scalar.activation(
                        out=ysb, in_=ps,
                        func=mybir.ActivationFunctionType.Identity,
                        bias=bo_sb[:, m:m + 1],
                    )
                    nc.sync.dma_start(out=yT_d.ap()[s, :, m, :], in_=ysb)
                    yield

            def drive(gen, n=1):
                if gen is None:
                    return False
                for _ in range(n):
                    try:
                        next(gen)
                    except StopIteration:
                        return False
                return True

            def drain(*gens):
                for g in gens:
                    while drive(g):
                        pass

            # ---- pipelined schedule ----
            load_x(0)
            a0, b0 = phase_a(0), phase_b(0)
            drain(a0, b0)

            load_x(1)
            c0, a1, b1 = phase_c(0), phase_a(1), phase_b(1)
            for _ in range(HP):          # 8 hp steps ; 16 A-steps ; 8 B-steps
                drive(c0)
                drive(a1, 2)
                drive(b1, 1)
            drain(c0, a1, b1)

            d0, c1 = phase_d(0), phase_c(1)
            for _ in range(HP):
                drive(c1)
                drive(d0, 1)
            drain(d0, c1)

            drain(phase_d(1))

    nc.compile()
    return nc


def _host_prep(x, w_qkv, rel_emb, w_out, b_out):
    """Build per-core input maps (bf16 casts, transposes, packing)."""
    bf = np.float16
    scale = DH ** -0.5

    xf = np.asarray(x, np.float32).reshape(SEQS, L, D)
    w_qkv = np.asarray(w_qkv, np.float32)
    rel_emb = np.asarray(rel_emb, np.float32)
    w_out = np.asarray(w_out, np.float32)
    b_out = np.asarray(b_out, np.float32)

    # xT: [seq, 128, KC, L]  (element [p, k, l] = x[seq, l, 128k+p])
    xT = xf.transpose(0, 2, 1).reshape(SEQS, KC, 128, L).transpose(0, 2, 1, 3)
    xT = np.ascontiguousarray(xT).astype(bf)

    # wqk: q columns pre-scaled; pack [m, p, k, c] = w[128k+p, 128m+c]
    wqk = w_qkv[:, :2 * D].copy()
    wqk[:, :D] *= scale
    wqk_p = wqk.reshape(KC, 128, 16, 128).transpose(2, 1, 0, 3)
    wqk_p = np.ascontiguousarray(wqk_p).astype(bf)

    # wv: [n, p, k, c] = w_v[128k+p, 512n+c]
    wv = w_qkv[:, 2 * D:]
    wv_p = wv.reshape(KC, 128, 2, 512).transpose(2, 1, 0, 3)
    wv_p = np.ascontiguousarray(wv_p).astype(bf)

    # wo: [i, p, m, c] = w_out[128i+p, 128m+c]
    wo_p = w_out.reshape(KC, 128, 8, 128)
    wo_p = np.ascontiguousarray(wo_p).astype(bf)

    # expb skewed tiles: expb[h, p, u] = exp(g_h[u - p - 384]),
    # g_h[d] = rel_emb[clip(d, -127, 127) + 127, h]
    u = np.arange(EXPB_W)[None, :]
    p = np.arange(128)[:, None]
    didx = np.clip(u - p - 384, -(MAX_REL - 1), MAX_REL - 1) + (MAX_REL - 1)
    expb = np.exp(rel_emb[didx, :]).transpose(2, 0, 1)  # [h, 128, 896]
    expb = np.ascontiguousarray(expb).astype(bf)

    # b_out packed [p, m] = b_out[128m + p]
    bo_p = np.ascontiguousarray(b_out.reshape(8, 128).T).astype(np.float32)

    shared = {
        "wqk": wqk_p, "wv": wv_p, "wo": wo_p, "expb": expb, "bo": bo_p,
    }
    in_maps = []
    for c in range(N_CORES):
        m = dict(shared)
        m["xT"] = xT[c * SPC:(c + 1) * SPC]
        in_maps.append(m)
    return in_maps


_PROGRAM = None


def kernel(x, w_qkv, rel_emb, w_out, b_out):
    global _PROGRAM, LAST_EXEC_TIME_NS
    if _PROGRAM is None:
        _PROGRAM = _build_program()
    nc = _PROGRAM

    in_maps = _host_prep(x, w_qkv, rel_emb, w_out, b_out)
    trace = bool(int(os.environ.get("TRN_KERNEL_TRACE", "0")))
    res = bass_utils.run_bass_kernel_spmd(
        nc, in_maps, core_ids=list(range(N_CORES)), trace=trace,
    )
    LAST_EXEC_TIME_NS = res.exec_time_ns

    # gather: yT [SPC, 128, 8, L] per core -> y [B, T, L, D]
    y = np.empty((SEQS, L, D), np.float32)
    for c in range(N_CORES):
        yT = np.asarray(res.results[c]["yT"], np.float32)
        for s in range(SPC):
            # [128, 8, L] -> [D, L] -> [L, D]
            y[c * SPC + s] = yT[s].reshape(128, 8, L).transpose(1, 0, 2).reshape(D, L).T
    return y.reshape(B, T, L, D)



# revision 2
# speedup vs baseline: 1.1644x; 1.1644x over previous
"""Trainium2 Bass kernel: attention with relative-position bias (v2).

Reference computation (per sequence, B*T=16 sequences of L=512, D=1024):
    qkv = x @ w_qkv;  q,k,v split;  S = q k^T / sqrt(dh) + rel_bias
    P = softmax(S);   out = (P @ v) @ w_out + b_out

Sharding: data-parallel over the B*T axis - 2 sequences per NeuronCore,
weights replicated. No collectives.

v2 redesign vs v1 (which measured ~490us with TensorE only ~59% busy and
8-15us PE gaps in the second half):
  - Attention (phase C) restructured per head-pair with S tiles batched
    [128,1024] across two r-chunks (2 PSUM banks each, sA/sB for the two
    heads), so each ACT exp instruction covers 1024 elems (amortizes the
    ~350-cycle ACT startup) and the two heads' QK matmuls run concurrently
    via tile_position row packing.
  - Per-pair softmax chain (QK -> exp -> mul -> PV -> normalize) is
    software-pipelined against projection matmuls of the *other* phases;
    phase A of seq 1 and B of seq 1 fill seq-0's attention, D of seq 0
    fills seq-1's attention, keeping the PE warm (HAM K=8/8).
  - A-phase PSUM evacuations moved to ScalarE (Copy), B-phase strided
    evacuation as one [128,1024] DVE op per lc chunk, bias multiplies
    split DVE (3/4) / GpSimd (1/4), normalize = DVE reciprocal direct
    from PSUM + GpSimd partition_broadcast + DVE multiply.
  - PSUM plan: sA,sB = 2x2 banks, po0,po1 = 2 banks, A/B/D accum = 2
    banks (8 total).
  - Output stored f16 (cast to f32 on host) to halve output DMA.
"""

import os
import numpy as np

import concourse.bass as bass
import concourse.mybir as mybir
import concourse.tile as tile
from concourse import bacc, bass_utils

HEADS = 16
MAX_REL = 128
B, T, L, D = 2, 8, 512, 1024
DH = D // HEADS          # 64
N_CORES = 8
SEQS = B * T             # 16
SPC = SEQS // N_CORES    # sequences per core = 2
KC = D // 128            # contraction chunks = 8
LC = L // 128            # sequence chunks = 4
HP = HEADS // 2          # head pairs = 8
EXPB_W = 896             # skewed bias tile width (512 + 3*128)

_F32 = mybir.dt.float32
_F16 = mybir.dt.float16

LAST_EXEC_TIME_NS = None


def _build_program():
    nc = bacc.Bacc("TRN2", debug=False)

    xT_d = nc.dram_tensor("xT", [SPC, 128, KC, L], _F16, kind="ExternalInput")
    wqk_d = nc.dram_tensor("wqk", [16, 128, KC, 128], _F16, kind="ExternalInput")
    wv_d = nc.dram_tensor("wv", [2, 128, KC, 512], _F16, kind="ExternalInput")
    wo_d = nc.dram_tensor("wo", [KC, 128, 8, 128], _F16, kind="ExternalInput")
    expb_d = nc.dram_tensor("expb", [HEADS, 128, EXPB_W], _F16, kind="ExternalInput")
    bo_d = nc.dram_tensor("bo", [128, 8], _F32, kind="ExternalInput")
    yT_d = nc.dram_tensor("yT", [SPC, 128, 8, L], _F16, kind="ExternalOutput")

    with tile.TileContext(nc) as tc:
        with (
            tc.tile_pool(name="const", bufs=1) as const_pool,
            tc.tile_pool(name="wstream", bufs=4) as wstream,
            tc.tile_pool(name="xt", bufs=2) as xt_pool,
            tc.tile_pool(name="qkt", bufs=2) as qkt_pool,
            tc.tile_pool(name="vaug", bufs=2) as vaug_pool,
            tc.tile_pool(name="ptile", bufs=2) as p_pool,
            tc.tile_pool(name="ot", bufs=2) as ot_pool,
            tc.tile_pool(name="norm", bufs=3) as norm_pool,
            tc.tile_pool(name="ysb", bufs=3) as y_pool,
            tc.tile_pool(name="ps_mm", bufs=2, space="PSUM") as ps_mm,
            tc.tile_pool(name="ps_s", bufs=1, space="PSUM") as ps_s,
            tc.tile_pool(name="ps_o", bufs=1, space="PSUM") as ps_o,
        ):
            # ---- constants (SWDGE queue via gpsimd) ----
            wv_sb = const_pool.tile([128, 2, KC, 512], _F16)
            nc.gpsimd.dma_start(out=wv_sb, in_=wv_d.ap().rearrange("n p k c -> p n k c"))
            expb_sb = const_pool.tile([128, HEADS, EXPB_W], _F16)
            nc.gpsimd.dma_start(
                out=expb_sb, in_=expb_d.ap().rearrange("h p u -> p h u")
            )
            wo_sb = const_pool.tile([128, KC, 8, 128], _F16)
            nc.gpsimd.dma_start(out=wo_sb, in_=wo_d.ap().rearrange("i p m c -> p i m c"))
            bo_sb = const_pool.tile([128, 8], _F32)
            nc.gpsimd.dma_start(out=bo_sb, in_=bo_d.ap())

            # prime the ACT exp table set early (one-time ~2.7us load)
            prime = norm_pool.tile([1, 16], _F32, name="prime", tag="prime")
            nc.vector.memset(prime, 0.0)
            prime_o = norm_pool.tile([1, 16], _F16, name="prime_o", tag="prime_o")
            nc.scalar.activation(
                out=prime_o, in_=prime, func=mybir.ActivationFunctionType.Exp
            )

            xt_sb = [None] * SPC
            qkt = [None] * SPC
            vaug = [None] * SPC
            ot = [None] * SPC

            def load_x(s):
                xt_sb[s] = xt_pool.tile([128, KC, L], _F16, name="xt", tag="xt")
                nc.sync.dma_start(out=xt_sb[s], in_=xT_d.ap()[s])

            def phase_a(s):
                """q/k projection, 8 steps; step j computes m-chunks j and 8+j
                so head-pair j is attention-ready after step j."""
                qkt[s] = qkt_pool.tile([128, 16, L], _F16, name="qkt", tag="qkt")
                for j in range(8):
                    for m in (j, 8 + j):
                        wqk_sb = wstream.tile([128, KC, 128], _F16, name="wqk", tag="wqk")
                        nc.sync.dma_start(out=wqk_sb, in_=wqk_d.ap()[m])
                        ps = ps_mm.tile([128, L], _F32, name="ps", tag="ps")
                        for k in range(KC):
                            nc.tensor.matmul(
                                ps,
                                wqk_sb[:, k, :],
                                xt_sb[s][:, k, :],
                                start=(k == 0),
                                stop=(k == KC - 1),
                            )
                        nc.scalar.activation(
                            out=qkt[s][:, m, :], in_=ps,
                            func=mybir.ActivationFunctionType.Copy,
                        )
                    yield

            def phase_b(s):
                """v projection: 4 lc steps (2 yields each)."""
                vaug[s] = vaug_pool.tile([128, LC, HEADS * 65], _F16, name="vaug", tag="vaug")
                va = vaug[s]
                # ones columns (col 64 of each 65-group) for the softmax sums
                ones_dst = bass.AP(
                    tensor=va.tensor,
                    offset=va.offset + 64,
                    ap=[va.ap[0], [HEADS * 65, LC], [65, HEADS], [1, 1]],
                )
                nc.vector.memset(ones_dst, 1.0)
                for lc in range(LC):
                    ps0v = ps_mm.tile([128, 512], _F32, name="ps0v", tag="ps")
                    ps1v = ps_mm.tile([128, 512], _F32, name="ps1v", tag="ps")
                    for k in range(KC):
                        nc.tensor.matmul(
                            ps0v,
                            xt_sb[s][:, k, lc * 128:(lc + 1) * 128],
                            wv_sb[:, 0, k, :],
                            start=(k == 0),
                            stop=(k == KC - 1),
                        )
                        nc.tensor.matmul(
                            ps1v,
                            xt_sb[s][:, k, lc * 128:(lc + 1) * 128],
                            wv_sb[:, 1, k, :],
                            start=(k == 0),
                            stop=(k == KC - 1),
                        )
                    yield
                    for nh, ps in ((0, ps0v), (1, ps1v)):
                        dst = bass.AP(
                            tensor=va.tensor,
                            offset=va.offset + lc * (HEADS * 65) + nh * 8 * 65,
                            ap=[va.ap[0], [65, 8], [1, 64]],
                        )
                        nc.vector.tensor_copy(
                            out=dst, in_=ps.rearrange("p (h c) -> p h c", h=8)
                        )
                    yield

            def phase_c(s):
                """attention: 8 head-pair steps, 4 yields each.

                Per pair: S computed into two [128,1024] PSUM tiles (sA=h0,
                sB=h1; two r-chunks per tile), exp batched per tile, bias
                multiply per r-chunk (DVE for h0 + r01 of h1, GpSimd for r23
                of h1), PV accumulated per head with the ones-column sums,
                then normalize via reciprocal + partition broadcast.
                """
                ot[s] = ot_pool.tile([128, KC, L], _F16, name="ot", tag="ot")
                for hp in range(HP):
                    h0, h1 = 2 * hp, 2 * hp + 1
                    q_tile = qkt[s][:, hp, :]
                    k_tile = qkt[s][:, 8 + hp, :]
                    p0 = p_pool.tile([128, LC, 512], _F16, name="p0", tag="p0")
                    p1 = p_pool.tile([128, LC, 512], _F16, name="p1", tag="p1")
                    for half in range(2):
                        r0 = 2 * half
                        sA = ps_s.tile([128, 1024], _F32, name="sA", tag="sA")
                        sB = ps_s.tile([128, 1024], _F32, name="sB", tag="sB")
                        for i, r in enumerate((r0, r0 + 1)):
                            nc.tensor.matmul(
                                sA[:, i * 512:(i + 1) * 512],
                                k_tile[0:64, r * 128:(r + 1) * 128],
                                q_tile[0:64, :],
                                start=True, stop=True,
                            )
                            nc.tensor.matmul(
                                sB[:, i * 512:(i + 1) * 512],
                                k_tile[64:128, r * 128:(r + 1) * 128],
                                q_tile[64:128, :],
                                start=True, stop=True,
                                tile_position=(64, 0),
                            )
                        nc.scalar.activation(
                            out=p0[:, r0:r0 + 2, :], in_=sA,
                            func=mybir.ActivationFunctionType.Exp,
                        )
                        nc.scalar.activation(
                            out=p1[:, r0:r0 + 2, :], in_=sB,
                            func=mybir.ActivationFunctionType.Exp,
                        )
                        for r in (r0, r0 + 1):
                            off = 384 - 128 * r
                            nc.vector.tensor_mul(
                                out=p0[:, r, :], in0=p0[:, r, :],
                                in1=expb_sb[:, h0, off:off + 512],
                            )
                            mul_eng = nc.gpsimd if half == 1 else nc.vector
                            mul_eng.tensor_mul(
                                out=p1[:, r, :], in0=p1[:, r, :],
                                in1=expb_sb[:, h1, off:off + 512],
                            )
                        yield

                    po0 = ps_o.tile([65, 512], _F32, name="po0", tag="po0")
                    po1 = ps_o.tile([65, 512], _F32, name="po1", tag="po1")
                    for r in range(LC):
                        nc.tensor.matmul(
                            po0,
                            vaug[s][:, r, h0 * 65:h0 * 65 + 65],
                            p0[:, r, :],
                            start=(r == 0), stop=(r == LC - 1),
                        )
                        nc.tensor.matmul(
                            po1,
                            vaug[s][:, r, h1 * 65:h1 * 65 + 65],
                            p1[:, r, :],
                            start=(r == 0), stop=(r == LC - 1),
                        )
                    yield
                    for idx, po in ((0, po0), (1, po1)):
                        rsum = norm_pool.tile([1, 512], _F32, name="rsum", tag="rsum")
                        nc.vector.reciprocal_approx_fast(out=rsum, in_=po[64:65, :])
                        rb = norm_pool.tile([64, 512], _F32, name="rb", tag="rb")
                        nc.gpsimd.partition_broadcast(rb, rsum)
                        nc.vector.tensor_mul(
                            out=ot[s][idx * 64:(idx + 1) * 64, hp, :],
                            in0=po[0:64, :],
                            in1=rb,
                        )
                    yield

            def phase_d(s):
                """output projection: 8 m-chunk steps."""
                for m in range(8):
                    ps = ps_mm.tile([128, L], _F32, name="ps", tag="ps")
                    for i in range(KC):
                        nc.tensor.matmul(
                            ps,
                            wo_sb[:, i, m, :],
                            ot[s][:, i, :],
                            start=(i == 0),
                            stop=(i == KC - 1),
                        )
                    ysb = y_pool.tile([128, L], _F16, name="ysb", tag="ysb")
                    nc.scalar.activation(
                        out=ysb, in_=ps,
                        func=mybir.ActivationFunctionType.Identity,
                        bias=bo_sb[:, m:m + 1],
                    )
                    nc.sync.dma_start(out=yT_d.ap()[s, :, m, :], in_=ysb)
                    yield

            def drive(gen, n=1):
                if gen is None:
                    return False
                for _ in range(n):
                    try:
                        next(gen)
                    except StopIteration:
                        return False
                return True

            def drain(*gens):
                for g in gens:
                    while drive(g):
                        pass

            # ---- pipelined schedule ----
            load_x(0)
            a0, b0 = phase_a(0), phase_b(0)
            drive(a0, 2)          # pairs 0,1 ready
            drain(b0)             # v(0) complete before C(0) PV
            c0 = phase_c(0)
            for _ in range(6):    # a0 steps 2..7; c0 18/32 yields
                drive(a0)
                drive(c0, 3)
            load_x(1)
            a1, b1 = phase_a(1), phase_b(1)
            for _ in range(8):    # b1; c0 +16 = 34
                drive(b1)
                drive(c0, 2)
            for _ in range(8):    # a1; c0 +24 = 58
                drive(a1)
                drive(c0, 3)
            drain(c0, a1, b1)
            c1, d0 = phase_c(1), phase_d(0)
            for _ in range(8):    # d0; c1 8*4 = 32 yields
                drive(d0)
                drive(c1, 4)
            drain(c1, d0)
            drain(phase_d(1))

    nc.compile()
    return nc


def _host_prep(x, w_qkv, rel_emb, w_out, b_out):
    """Build per-core input maps (f16 casts, transposes, packing)."""
    bf = np.float16
    scale = DH ** -0.5

    xf = np.asarray(x, np.float32).reshape(SEQS, L, D)
    w_qkv = np.asarray(w_qkv, np.float32)
    rel_emb = np.asarray(rel_emb, np.float32)
    w_out = np.asarray(w_out, np.float32)
    b_out = np.asarray(b_out, np.float32)

    # xT: [seq, 128, KC, L]  (element [p, k, l] = x[seq, l, 128k+p])
    xT = xf.transpose(0, 2, 1).reshape(SEQS, KC, 128, L).transpose(0, 2, 1, 3)
    xT = np.ascontiguousarray(xT).astype(bf)

    # wqk: q columns pre-scaled; pack [m, p, k, c] = w[128k+p, 128m+c]
    wqk = w_qkv[:, :2 * D].copy()
    wqk[:, :D] *= scale
    wqk_p = wqk.reshape(KC, 128, 16, 128).transpose(2, 1, 0, 3)
    wqk_p = np.ascontiguousarray(wqk_p).astype(bf)

    # wv: [n, p, k, c] = w_v[128k+p, 512n+c]
    wv = w_qkv[:, 2 * D:]
    wv_p = wv.reshape(KC, 128, 2, 512).transpose(2, 1, 0, 3)
    wv_p = np.ascontiguousarray(wv_p).astype(bf)

    # wo: [i, p, m, c] = w_out[128i+p, 128m+c]
    wo_p = w_out.reshape(KC, 128, 8, 128)
    wo_p = np.ascontiguousarray(wo_p).astype(bf)

    # expb skewed tiles: expb[h, p, u] = exp(g_h[u - p - 384]),
    # g_h[d] = rel_emb[clip(d, -127, 127) + 127, h]
    u = np.arange(EXPB_W)[None, :]
    p = np.arange(128)[:, None]
    didx = np.clip(u - p - 384, -(MAX_REL - 1), MAX_REL - 1) + (MAX_REL - 1)
    expb = np.exp(rel_emb[didx, :]).transpose(2, 0, 1)  # [h, 128, 896]
    expb = np.ascontiguousarray(expb).astype(bf)

    # b_out packed [p, m] = b_out[128m + p]
    bo_p = np.ascontiguousarray(b_out.reshape(8, 128).T).astype(np.float32)

    shared = {
        "wqk": wqk_p, "wv": wv_p, "wo": wo_p, "expb": expb, "bo": bo_p,
    }
    in_maps = []
    for c in range(N_CORES):
        m = dict(shared)
        m["xT"] = xT[c * SPC:(c + 1) * SPC]
        in_maps.append(m)
    return in_maps


_PROGRAM = None


def kernel(x, w_qkv, rel_emb, w_out, b_out):
    global _PROGRAM, LAST_EXEC_TIME_NS
    if _PROGRAM is None:
        _PROGRAM = _build_program()
    nc = _PROGRAM

    in_maps = _host_prep(x, w_qkv, rel_emb, w_out, b_out)
    trace = bool(int(os.environ.get("TRN_KERNEL_TRACE", "0")))
    res = bass_utils.run_bass_kernel_spmd(
        nc, in_maps, core_ids=list(range(N_CORES)), trace=trace,
    )
    LAST_EXEC_TIME_NS = res.exec_time_ns

    # gather: yT [SPC, 128, 8, L] f16 per core -> y [B, T, L, D] f32
    y = np.empty((SEQS, L, D), np.float32)
    for c in range(N_CORES):
        yT = np.asarray(res.results[c]["yT"], np.float32)
        for s in range(SPC):
            # [128, 8, L] -> [D, L] -> [L, D]
            y[c * SPC + s] = yT[s].reshape(128, 8, L).transpose(1, 0, 2).reshape(D, L).T
    return y.reshape(B, T, L, D)


# revision 9
# speedup vs baseline: 1.3614x; 1.1692x over previous
"""Trainium2 Bass kernel: attention with relative-position bias (v2).

Reference computation (per sequence, B*T=16 sequences of L=512, D=1024):
    qkv = x @ w_qkv;  q,k,v split;  S = q k^T / sqrt(dh) + rel_bias
    P = softmax(S);   out = (P @ v) @ w_out + b_out

Sharding: data-parallel over the B*T axis - 2 sequences per NeuronCore,
weights replicated. No collectives.

v2 redesign vs v1 (which measured ~490us with TensorE only ~59% busy and
8-15us PE gaps in the second half):
  - Attention (phase C) restructured per head-pair with S tiles batched
    [128,1024] across two r-chunks (2 PSUM banks each, sA/sB for the two
    heads), so each ACT exp instruction covers 1024 elems (amortizes the
    ~350-cycle ACT startup) and the two heads' QK matmuls run concurrently
    via tile_position row packing.
  - Per-pair softmax chain (QK -> exp -> mul -> PV -> normalize) is
    software-pipelined against projection matmuls of the *other* phases;
    phase A of seq 1 and B of seq 1 fill seq-0's attention, D of seq 0
    fills seq-1's attention, keeping the PE warm (HAM K=8/8).
  - A-phase PSUM evacuations moved to ScalarE (Copy), B-phase strided
    evacuation as one [128,1024] DVE op per lc chunk, bias multiplies
    split DVE (3/4) / GpSimd (1/4), normalize = DVE reciprocal direct
    from PSUM + GpSimd partition_broadcast + DVE multiply.
  - PSUM plan: sA,sB = 2x2 banks, po0,po1 = 2 banks, A/B/D accum = 2
    banks (8 total).
  - Output stored f16 (cast to f32 on host) to halve output DMA.
"""

import os
import numpy as np

import concourse.bass as bass
import concourse.mybir as mybir
import concourse.tile as tile
from concourse import bacc, bass_utils

HEADS = 16
MAX_REL = 128
B, T, L, D = 2, 8, 512, 1024
DH = D // HEADS          # 64
N_CORES = 8
SEQS = B * T             # 16
SPC = SEQS // N_CORES    # sequences per core = 2
KC = D // 128            # contraction chunks = 8
LC = L // 128            # sequence chunks = 4
HP = HEADS // 2          # head pairs = 8
EXPB_W = 896             # skewed bias tile width (512 + 3*128)

_F32 = mybir.dt.float32
_F16 = mybir.dt.float16

LAST_EXEC_TIME_NS = None


def _build_program():
    nc = bacc.Bacc("TRN2", debug=False)

    xT_d = nc.dram_tensor("xT", [SPC, 128, KC, L], _F16, kind="ExternalInput")
    wqk_d = nc.dram_tensor("wqk", [16, 128, KC, 128], _F16, kind="ExternalInput")
    wv_d = nc.dram_tensor("wv", [2, 128, KC, 512], _F16, kind="ExternalInput")
    wo_d = nc.dram_tensor("wo", [KC, 128, 8, 128], _F16, kind="ExternalInput")
    expb_d = nc.dram_tensor("expb", [HEADS, 128, EXPB_W], _F16, kind="ExternalInput")
    bo_d = nc.dram_tensor("bo", [128, 8], _F32, kind="ExternalInput")
    yT_d = nc.dram_tensor("yT", [SPC, 128, 8, L], _F16, kind="ExternalOutput")

    with tile.TileContext(nc) as tc:
        with (
            tc.tile_pool(name="const", bufs=1) as const_pool,
            tc.tile_pool(name="wstream", bufs=4) as wstream,
            tc.tile_pool(name="xt", bufs=2) as xt_pool,
            tc.tile_pool(name="qkt", bufs=2) as qkt_pool,
            tc.tile_pool(name="vaug", bufs=2) as vaug_pool,
            tc.tile_pool(name="ptile", bufs=2) as p_pool,
            tc.tile_pool(name="ot", bufs=2) as ot_pool,
            tc.tile_pool(name="norm", bufs=3) as norm_pool,
            tc.tile_pool(name="ysb", bufs=3) as y_pool,
            tc.tile_pool(name="osb", bufs=4) as osb_pool,
            tc.tile_pool(name="ps_mm", bufs=1, space="PSUM") as ps_mm,
            tc.tile_pool(name="ps_s", bufs=1, space="PSUM") as ps_s,
            tc.tile_pool(name="ps_o", bufs=1, space="PSUM") as ps_o,
        ):
            # ---- constants (SWDGE queue via gpsimd) ----
            wv_sb = const_pool.tile([128, 2, KC, 512], _F16)
            nc.gpsimd.dma_start(out=wv_sb, in_=wv_d.ap().rearrange("n p k c -> p n k c"))
            expb_sb = const_pool.tile([128, HEADS, EXPB_W], _F16)
            nc.gpsimd.dma_start(
                out=expb_sb, in_=expb_d.ap().rearrange("h p u -> p h u")
            )
            wo_sb = const_pool.tile([128, KC, 8, 128], _F16)
            nc.gpsimd.dma_start(out=wo_sb, in_=wo_d.ap().rearrange("i p m c -> p i m c"))
            bo_sb = const_pool.tile([128, 8], _F32)
            nc.gpsimd.dma_start(out=bo_sb, in_=bo_d.ap())

            # prime the ACT exp table set early (one-time ~2.7us load)
            prime = norm_pool.tile([1, 16], _F32, name="prime", tag="prime")
            nc.vector.memset(prime, 0.0)
            prime_o = norm_pool.tile([1, 16], _F16, name="prime_o", tag="prime_o")
            nc.scalar.activation(
                out=prime_o, in_=prime, func=mybir.ActivationFunctionType.Exp
            )

            xt_sb = [None] * SPC
            qkt = [None] * SPC
            vaug = [None] * SPC
            ot = [None] * SPC

            def load_x(s):
                xt_sb[s] = xt_pool.tile([128, KC, L], _F16, name="xt", tag="xt")
                nc.sync.dma_start(out=xt_sb[s], in_=xT_d.ap()[s])

            def phase_a(s):
                """q/k projection, 8 steps; step j computes m-chunks j and 8+j
                so head-pair j is attention-ready after step j."""
                qkt[s] = qkt_pool.tile([128, 16, L], _F16, name="qkt", tag="qkt")
                for j in range(8):
                    ps = ps_mm.tile([128, 1024], _F32, name="ps", tag="ps")
                    for half, m in ((0, j), (1, 8 + j)):
                        wqk_sb = wstream.tile([128, KC, 128], _F16, name="wqk", tag="wqk")
                        nc.sync.dma_start(out=wqk_sb, in_=wqk_d.ap()[m])
                        for k in range(KC):
                            nc.tensor.matmul(
                                ps[:, half * 512:(half + 1) * 512],
                                wqk_sb[:, k, :],
                                xt_sb[s][:, k, :],
                                start=(k == 0),
                                stop=(k == KC - 1),
                            )
                    # one strided evacuation into chunks j and 8+j
                    qk = qkt[s]
                    dst = bass.AP(
                        tensor=qk.tensor,
                        offset=qk.offset + j * 512,
                        ap=[qk.ap[0], [8 * 512, 2], [1, 512]],
                    )
                    nc.scalar.activation(
                        out=dst, in_=ps,
                        func=mybir.ActivationFunctionType.Copy,
                    )
                    yield

            def phase_b(s):
                """v projection: 4 lc steps (2 yields each)."""
                vaug[s] = vaug_pool.tile([128, LC, HEADS * 65], _F16, name="vaug", tag="vaug")
                va = vaug[s]
                # ones columns (col 64 of each 65-group) for the softmax sums
                ones_dst = bass.AP(
                    tensor=va.tensor,
                    offset=va.offset + 64,
                    ap=[va.ap[0], [HEADS * 65, LC], [65, HEADS], [1, 1]],
                )
                nc.vector.memset(ones_dst, 1.0)
                for lc in range(LC):
                    psv = ps_mm.tile([128, 1024], _F32, name="psv", tag="ps")
                    for k in range(KC):
                        nc.tensor.matmul(
                            psv[:, 0:512],
                            xt_sb[s][:, k, lc * 128:(lc + 1) * 128],
                            wv_sb[:, 0, k, :],
                            start=(k == 0),
                            stop=(k == KC - 1),
                        )
                        nc.tensor.matmul(
                            psv[:, 512:1024],
                            xt_sb[s][:, k, lc * 128:(lc + 1) * 128],
                            wv_sb[:, 1, k, :],
                            start=(k == 0),
                            stop=(k == KC - 1),
                        )
                    yield
                    dst = bass.AP(
                        tensor=va.tensor,
                        offset=va.offset + lc * (HEADS * 65),
                        ap=[va.ap[0], [65, 16], [1, 64]],
                    )
                    nc.vector.tensor_copy(
                        out=dst, in_=psv.rearrange("p (h c) -> p h c", h=16)
                    )
                    yield

            def phase_c(s):
                """attention: 8 head-pair steps, 4 yields each.

                Per pair: S computed into two [128,1024] PSUM tiles (sA=h0,
                sB=h1; two r-chunks per tile), exp batched per tile, bias
                multiply per r-chunk (DVE for h0 + r01 of h1, GpSimd for r23
                of h1), PV accumulated per head with the ones-column sums,
                then normalize via reciprocal + partition broadcast.
                """
                ot[s] = ot_pool.tile([128, KC, L], _F16, name="ot", tag="ot")
                for hp in range(HP):
                    h0, h1 = 2 * hp, 2 * hp + 1
                    q_tile = qkt[s][:, hp, :]
                    k_tile = qkt[s][:, 8 + hp, :]
                    # pp[:, hh, r, :] = exp(S^T) for head 2hp+hh, key chunk r
                    pp = p_pool.tile([128, 2, LC, 512], _F16, name="pp", tag="pp")
                    for half in range(2):
                        r0 = 2 * half
                        sS = ps_s.tile([128, 2048], _F32, name="sS", tag="sS")
                        for i, r in enumerate((r0, r0 + 1)):
                            nc.tensor.matmul(
                                sS[:, i * 512:(i + 1) * 512],
                                k_tile[0:64, r * 128:(r + 1) * 128],
                                q_tile[0:64, :],
                                start=True, stop=True,
                            )
                            nc.tensor.matmul(
                                sS[:, 1024 + i * 512:1024 + (i + 1) * 512],
                                k_tile[64:128, r * 128:(r + 1) * 128],
                                q_tile[64:128, :],
                                start=True, stop=True,
                                tile_position=(64, 0),
                            )
                        nc.scalar.activation(
                            out=pp[:, :, r0:r0 + 2, :], in_=sS,
                            func=mybir.ActivationFunctionType.Exp,
                        )
                        for r in (r0, r0 + 1):
                            off = 384 - 128 * r
                            nc.vector.tensor_mul(
                                out=pp[:, 0, r, :], in0=pp[:, 0, r, :],
                                in1=expb_sb[:, h0, off:off + 512],
                            )
                            mul_eng = nc.gpsimd if half == 1 else nc.vector
                            mul_eng.tensor_mul(
                                out=pp[:, 1, r, :], in0=pp[:, 1, r, :],
                                in1=expb_sb[:, h1, off:off + 512],
                            )
                        yield

                    po0 = ps_o.tile([65, 512], _F32, name="po0", tag="po0")
                    po1 = ps_o.tile([65, 512], _F32, name="po1", tag="po1")
                    for r in range(LC):
                        nc.tensor.matmul(
                            po0,
                            vaug[s][:, r, h0 * 65:h0 * 65 + 65],
                            pp[:, 0, r, :],
                            start=(r == 0), stop=(r == LC - 1),
                        )
                        nc.tensor.matmul(
                            po1,
                            vaug[s][:, r, h1 * 65:h1 * 65 + 65],
                            pp[:, 1, r, :],
                            start=(r == 0), stop=(r == LC - 1),
                        )
                    yield
                    # reciprocal + fast PSUM evacuation (f16) free the po banks
                    # quickly (both DVE, no cross-engine rendezvous); the
                    # broadcast + normalize multiply run lazily from SBUF.
                    o_sb = [None, None]
                    rsum = [None, None]
                    for idx, po in ((0, po0), (1, po1)):
                        rsum[idx] = norm_pool.tile([1, 512], _F32, name="rsum", tag="rsum")
                        nc.vector.reciprocal_approx_fast(out=rsum[idx], in_=po[64:65, :])
                        o_sb[idx] = osb_pool.tile(
                            [64, 512], _F16, name="o_sb", tag="o_sb"
                        )
                        nc.vector.tensor_copy(out=o_sb[idx], in_=po[0:64, :])
                    yield
                    for idx in range(2):
                        rb = norm_pool.tile([64, 512], _F32, name="rb", tag="rb")
                        nc.gpsimd.partition_broadcast(rb, rsum[idx])
                        nc.vector.tensor_mul(
                            out=ot[s][idx * 64:(idx + 1) * 64, hp, :],
                            in0=o_sb[idx][0:64, :],
                            in1=rb,
                        )
                    yield

            def phase_d(s):
                """output projection: 8 m-chunk steps."""
                ps = None
                for m in range(8):
                    half = m % 2
                    if half == 0:
                        ps = ps_mm.tile([128, 1024], _F32, name="ps", tag="ps")
                    for i in range(KC):
                        nc.tensor.matmul(
                            ps[:, half * 512:(half + 1) * 512],
                            wo_sb[:, i, m, :],
                            ot[s][:, i, :],
                            start=(i == 0),
                            stop=(i == KC - 1),
                        )
                    ysb = y_pool.tile([128, L], _F16, name="ysb", tag="ysb")
                    nc.scalar.activation(
                        out=ysb, in_=ps[:, half * 512:(half + 1) * 512],
                        func=mybir.ActivationFunctionType.Identity,
                        bias=bo_sb[:, m:m + 1],
                    )
                    nc.sync.dma_start(out=yT_d.ap()[s, :, m, :], in_=ysb)
                    yield

            def drive(gen, n=1):
                if gen is None:
                    return False
                for _ in range(n):
                    try:
                        next(gen)
                    except StopIteration:
                        return False
                return True

            def drain(*gens):
                for g in gens:
                    while drive(g):
                        pass

            # ---- pipelined schedule ----
            load_x(0)
            a0, b0 = phase_a(0), phase_b(0)
            drive(a0, 2)          # pairs 0,1 ready
            drain(b0)             # v(0) complete before C(0) PV
            c0 = phase_c(0)
            for _ in range(6):    # a0 steps 2..7; c0 18/40 yields
                drive(a0)
                drive(c0, 3)
            load_x(1)
            a1, b1 = phase_a(1), phase_b(1)
            for _ in range(8):    # b1; c0 +24 = 42
                drive(b1)
                drive(c0, 3)
            for _ in range(8):    # a1; c0 +32 -> drains
                drive(a1)
                drive(c0, 4)
            drain(c0, a1, b1)
            c1, d0 = phase_c(1), phase_d(0)
            for _ in range(8):    # d0; c1 8*5 = 40 yields
                drive(d0)
                drive(c1, 5)
            drain(c1, d0)
            drain(phase_d(1))

    nc.compile()
    return nc


def _host_prep(x, w_qkv, rel_emb, w_out, b_out):
    """Build per-core input maps (f16 casts, transposes, packing)."""
    bf = np.float16
    scale = DH ** -0.5

    xf = np.asarray(x, np.float32).reshape(SEQS, L, D)
    w_qkv = np.asarray(w_qkv, np.float32)
    rel_emb = np.asarray(rel_emb, np.float32)
    w_out = np.asarray(w_out, np.float32)
    b_out = np.asarray(b_out, np.float32)

    # xT: [seq, 128, KC, L]  (element [p, k, l] = x[seq, l, 128k+p])
    xT = xf.transpose(0, 2, 1).reshape(SEQS, KC, 128, L).transpose(0, 2, 1, 3)
    xT = np.ascontiguousarray(xT).astype(bf)

    # wqk: q columns pre-scaled; pack [m, p, k, c] = w[128k+p, 128m+c]
    wqk = w_qkv[:, :2 * D].copy()
    wqk[:, :D] *= scale
    wqk_p = wqk.reshape(KC, 128, 16, 128).transpose(2, 1, 0, 3)
    wqk_p = np.ascontiguousarray(wqk_p).astype(bf)

    # wv: [n, p, k, c] = w_v[128k+p, 512n+c]
    wv = w_qkv[:, 2 * D:]
    wv_p = wv.reshape(KC, 128, 2, 512).transpose(2, 1, 0, 3)
    wv_p = np.ascontiguousarray(wv_p).astype(bf)

    # wo: [i, p, m, c] = w_out[128i+p, 128m+c]
    wo_p = w_out.reshape(KC, 128, 8, 128)
    wo_p = np.ascontiguousarray(wo_p).astype(bf)

    # expb skewed tiles: expb[h, p, u] = exp(g_h[u - p - 384]),
    # g_h[d] = rel_emb[clip(d, -127, 127) + 127, h]
    u = np.arange(EXPB_W)[None, :]
    p = np.arange(128)[:, None]
    didx = np.clip(u - p - 384, -(MAX_REL - 1), MAX_REL - 1) + (MAX_REL - 1)
    expb = np.exp(rel_emb[didx, :]).transpose(2, 0, 1)  # [h, 128, 896]
    expb = np.ascontiguousarray(expb).astype(bf)

    # b_out packed [p, m] = b_out[128m + p]
    bo_p = np.ascontiguousarray(b_out.reshape(8, 128).T).astype(np.float32)

    shared = {
        "wqk": wqk_p, "wv": wv_p, "wo": wo_p, "expb": expb, "bo": bo_p,
    }
    in_maps = []
    for c in range(N_CORES):
        m = dict(shared)
        m["xT"] = xT[c * SPC:(c + 1) * SPC]
        in_maps.append(m)
    return in_maps


_PROGRAM = None


def kernel(x, w_qkv, rel_emb, w_out, b_out):
    global _PROGRAM, LAST_EXEC_TIME_NS
    if _PROGRAM is None:
        _PROGRAM = _build_program()
    nc = _PROGRAM

    in_maps = _host_prep(x, w_qkv, rel_emb, w_out, b_out)
    trace = bool(int(os.environ.get("TRN_KERNEL_TRACE", "0")))
    res = bass_utils.run_bass_kernel_spmd(
        nc, in_maps, core_ids=list(range(N_CORES)), trace=trace,
    )
    LAST_EXEC_TIME_NS = res.exec_time_ns

    # gather: yT [SPC, 128, 8, L] f16 per core -> y [B, T, L, D] f32
    y = np.empty((SEQS, L, D), np.float32)
    for c in range(N_CORES):
        yT = np.asarray(res.results[c]["yT"], np.float32)
        for s in range(SPC):
            # [128, 8, L] -> [D, L] -> [L, D]
            y[c * SPC + s] = yT[s].reshape(128, 8, L).transpose(1, 0, 2).reshape(D, L).T
    return y.reshape(B, T, L, D)
